# revision 12
# baseline (speedup 1.0000x reference)
"""Distributed self-attention kernel for Trainium2, 8 NeuronCores.

Head-parallel sharding (no collectives): with NH=16 heads on 8 cores,
each core owns one head PAIR (heads 2c, 2c+1 = hidden dims 128c..128c+128).
Every core loads the full x^T (6.3 MB bf16, column-chunked across DMA
queues so the first projection can start ~4us in), computes q/k/v
projections restricted to its pair's 128 output dims over the whole
sequence, runs attention for 2 heads x 3072 queries, and writes its
[3072, 128] slice of the hidden dim; the host concatenates.

The kernel is ScalarE-bound: exp() over 2x3072x3072 scores is ~153us of
ACT time that nothing else can absorb (DVE-side approximations fail the
accuracy budget: attention averaging passes per-element P noise through
to the output at full relative strength). Everything else is organized
to hide under that shadow:
  - scores run in transposed layout (s^T[key, query]); per-head k^T
    stationaries are zero-padded to the full 128-row contraction
    ([kE^T;0] / [0;kO^T]) so each head's scores are one full-rate matmul
    against the shared unpadded q block.
  - P@V stationary = a 128-col window of the per-key-tile v layout
    [v_even(64) | 1 | v_odd(64) | 1 | 0-pad(63)]; output rows 0-63 are
    context, row 64 the softmax denominator, rows 65-127 garbage.
  - exp on ScalarE, scale 1/sqrt(64) fused, no max subtraction (logits
    |qk/8| < ~4 — mathematically identical to the reference softmax).
  - the q/k/v projections AND the output transpose/normalize are
    interleaved into the attention sweep: key-tile groups are 2 wide so
    the score/ctx PSUM tiles leave 2 banks free (one rotating projection
    tile, one transpose tile). PSUM = 2+2+1+1+1+1 = 8 banks exactly.
Even/odd heads are staggered so ScalarE exp and PE matmuls pipeline.
"""

import numpy as np
import ml_dtypes

import concourse.bacc as bacc
import concourse.mybir as mybir
import concourse.tile as tile
from concourse import bass_utils

F32 = mybir.dt.float32
BF16 = mybir.dt.bfloat16
AF = mybir.ActivationFunctionType

N_CORES = 8
B, S, HID = 1, 3072, 1024
NH, HD = 16, 64
PD = 128                   # pair dims per core (2 heads x 64)
QC = 512                   # query chunk (moving cols per matmul)
NQC = S // QC              # 6 query chunks
KT = S // 128              # 24 key tiles
NG = KT // 2               # 12 groups of 2 key tiles
VTW = 193                  # per-kt v tile: vE(64)|1|vO(64)|1|zero-pad(63)
XCB = 768                  # x^T DMA column-chunk width

_cache: dict = {}


def _build(with_mask: bool, with_bias: bool):
    nc = bacc.Bacc("TRN2", target_bir_lowering=False, debug=False,
                   num_devices=N_CORES)

    J = 9 if with_bias else 8          # contraction slices (128 rows each)
    KIN = HID + 1 if with_bias else HID

    xt = nc.dram_tensor("xt", [KIN, S], BF16, kind="ExternalInput")
    w = nc.dram_tensor("w", [3, KIN, PD], BF16, kind="ExternalInput")
    ident = nc.dram_tensor("ident", [128, 128], F32, kind="ExternalInput")
    if with_mask:
        maskt = nc.dram_tensor("maskt", [128, KT], F32, kind="ExternalInput")
    out = nc.dram_tensor("out", [S, PD], F32, kind="ExternalOutput")

    with tile.TileContext(nc) as tc:
        with (
            tc.tile_pool(name="persist", bufs=1) as pp,
            tc.tile_pool(name="wpool", bufs=1) as wpool,
            tc.tile_pool(name="ppool", bufs=8) as ppool,
            tc.tile_pool(name="rpool", bufs=8) as rpool,
            # PSUM: 2+2+1+1+1+1 = 8 banks exactly
            tc.tile_pool(name="spoolE", bufs=1, space="PSUM") as spoolE,
            tc.tile_pool(name="spoolO", bufs=1, space="PSUM") as spoolO,
            tc.tile_pool(name="cpool", bufs=1, space="PSUM") as cpool,
            tc.tile_pool(name="pjpool", bufs=1, space="PSUM") as pjpool,
            tc.tile_pool(name="tpool", bufs=1, space="PSUM") as tpool,
        ):
            # ---- persistent SBUF tensors ----
            xsb = pp.tile([128, J * S], BF16)        # x^T contraction slices
            qsb = pp.tile([128, S], BF16)            # q^T pair block
            # k^T pair blocks, zero-padded per head so each head's scores
            # come out of one full-rate 128-contraction matmul:
            # ksbE = [kE^T; 0], ksbO = [0; kO^T]
            ksbE = pp.tile([128, S], BF16)
            ksbO = pp.tile([128, S], BF16)
            vsb = [pp.tile([128, VTW], BF16, name=f"vsb{k}") for k in range(KT)]
            idsb = pp.tile([128, 128], F32)
            ctxsb = [pp.tile([65, S], F32, name=f"ctxsb{h}") for h in range(2)]
            osb = [pp.tile([128, PD], F32, name=f"osb{t}") for t in range(KT)]

            def load_w(proj):
                wt = []
                for j in range(J):
                    if j < 8:
                        t = wpool.tile([128, PD], BF16, tag="w", bufs=24,
                                       name=f"w{proj}_{j}")
                        nc.sync.dma_start(t[:], w[proj, j * 128:(j + 1) * 128, :])
                    else:
                        t = wpool.tile([1, PD], BF16, tag="wb", bufs=3,
                                       name=f"wb{proj}")
                        nc.sync.dma_start(t[:], w[proj, HID:HID + 1, :])
                    wt.append(t)
                return wt

            # small DMAs first — they must not queue behind the 6.3MB x load
            wq, wk, wv = load_w(0), load_w(1), load_w(2)
            nc.sync.dma_start(idsb[:], ident[:])
            if with_mask:
                msb = pp.tile([128, KT], F32)
                nc.sync.dma_start(msb[:], maskt[:])

            # x^T load, column-chunked so early chunks land fast
            for cb in range(S // XCB):
                for j in range(8):
                    nc.sync.dma_start(
                        xsb[:, j * S + cb * XCB: j * S + (cb + 1) * XCB],
                        xt[j * 128:(j + 1) * 128, cb * XCB:(cb + 1) * XCB])
            if with_bias:
                nc.sync.dma_start(xsb[0:1, 8 * S:9 * S], xt[1024:1025, :])

            nc.vector.memset(ksbE[64:128, :], 0.0)
            nc.vector.memset(ksbO[0:64, :], 0.0)
            for k in range(KT):
                nc.vector.memset(vsb[k][:, 130:VTW], 0.0)
                nc.vector.memset(vsb[k][:, 64:65], 1.0)
                nc.vector.memset(vsb[k][:, 129:130], 1.0)

            # ---- projection emitters (interleaved into the qc sweep) ----
            def proj_chunk(wt, m, nm):
                # one 512-col chunk of k^T or q^T: [128 pair dims, 512 seq]
                pj = pjpool.tile([128, QC], F32, tag="pj", name=f"pj{nm}{m}")
                for j in range(J):
                    rows = 128 if j < 8 else 1
                    nc.tensor.matmul(
                        pj[:], wt[j][:rows, :],
                        xsb[:rows, j * S + m * QC: j * S + (m + 1) * QC],
                        start=(j == 0), stop=(j == J - 1))
                return pj

            def k_chunk(m):
                pj = proj_chunk(wk, m, "k")
                nc.vector.tensor_copy(ksbE[0:64, m * QC:(m + 1) * QC],
                                      pj[0:64, :])
                nc.vector.tensor_copy(ksbO[64:128, m * QC:(m + 1) * QC],
                                      pj[64:128, :])

            def q_chunk(m):
                pj = proj_chunk(wq, m, "q")
                nc.vector.tensor_copy(qsb[:, m * QC:(m + 1) * QC], pj[:])

            def v_quad(qd):
                # v in [key, dim] layout: 4 key-tiles of [128 keys, 128 dims]
                # accumulated side by side in one rotating PSUM bank
                vt = pjpool.tile([128, QC], F32, tag="pj", name=f"pjv{qd}")
                for ktl in range(4):
                    kt = qd * 4 + ktl
                    for j in range(J):
                        rows = 128 if j < 8 else 1
                        nc.tensor.matmul(
                            vt[:, ktl * 128:(ktl + 1) * 128],
                            xsb[:rows, j * S + kt * 128: j * S + (kt + 1) * 128],
                            wv[j][:rows, :],
                            start=(j == 0), stop=(j == J - 1))
                for ktl in range(4):
                    kt = qd * 4 + ktl
                    nc.vector.tensor_copy(
                        vsb[kt][:, 0:64], vt[:, ktl * 128: ktl * 128 + 64])
                    nc.vector.tensor_copy(
                        vsb[kt][:, 65:129], vt[:, ktl * 128 + 64: ktl * 128 + 128])

            # ---- attention sweep with A/D work woven in ----
            def score_block(sp, ksbh, qc, g):
                for j in range(2):
                    kt = g * 2 + j
                    nc.tensor.matmul(
                        sp[:, j * QC:(j + 1) * QC],
                        ksbh[:, kt * 128:(kt + 1) * 128],
                        qsb[:, qc * QC:(qc + 1) * QC],
                        start=True, stop=True)

            def exp_block(pt, sp, g):
                if with_mask:
                    for j in range(2):
                        kt = g * 2 + j
                        nc.scalar.activation(
                            pt[:, j * QC:(j + 1) * QC],
                            sp[:, j * QC:(j + 1) * QC], AF.Exp,
                            bias=msb[:, kt:kt + 1], scale=0.125)
                else:
                    nc.scalar.activation(pt[:], sp[:], AF.Exp, scale=0.125)

            def pv_block(ctx, pt, off, g):
                for j in range(2):
                    kt = g * 2 + j
                    nc.tensor.matmul(
                        ctx[:], vsb[kt][:, off:off + 128],
                        pt[:, j * QC:(j + 1) * QC],
                        start=(g == 0 and j == 0),
                        stop=(g == NG - 1 and j == 1))

            # prefix: first k / q chunks so the first exp fires early
            k_chunk(0)
            q_chunk(0)
            next_k, next_vq, next_q = 1, 0, 1

            for qc in range(NQC):
                ctxE = cpool.tile([128, QC], F32, tag="ctxE", name=f"cE{qc}")
                ctxO = cpool.tile([128, QC], F32, tag="ctxO", name=f"cO{qc}")
                for g in range(NG):
                    need = (2 * g + 1) // 4
                    while next_k <= need:
                        k_chunk(next_k)
                        next_k += 1
                    spE = spoolE.tile([128, 2 * QC], F32, tag="spE",
                                      name=f"spE{qc}_{g}")
                    score_block(spE, ksbE, qc, g)
                    ptE = ppool.tile([128, 2 * QC], BF16, tag="pt",
                                     name=f"ptE{qc}_{g}")
                    exp_block(ptE, spE, g)
                    spO = spoolO.tile([128, 2 * QC], F32, tag="spO",
                                      name=f"spO{qc}_{g}")
                    score_block(spO, ksbO, qc, g)
                    while next_vq <= need:
                        v_quad(next_vq)
                        next_vq += 1
                    ptO = ppool.tile([128, 2 * QC], BF16, tag="pt",
                                     name=f"ptO{qc}_{g}")
                    exp_block(ptO, spO, g)
                    pv_block(ctxE, ptE, 0, g)
                    pv_block(ctxO, ptO, 65, g)
                    if g == 8 and next_q <= qc + 1 and next_q < NQC:
                        q_chunk(next_q)
                        next_q += 1
                nc.vector.tensor_copy(ctxsb[0][:, qc * QC:(qc + 1) * QC],
                                      ctxE[0:65, :])
                nc.vector.tensor_copy(ctxsb[1][:, qc * QC:(qc + 1) * QC],
                                      ctxO[0:65, :])

                # ---- D: transpose back, normalize, store (this qc) ----
                for t in range(qc * 4, (qc + 1) * 4):
                    for h in range(2):
                        tp = tpool.tile([128, 65], F32, tag="tp",
                                        name=f"tp{h}_{t}")
                        nc.tensor.transpose(
                            tp[:], ctxsb[h][:, t * 128:(t + 1) * 128],
                            idsb[0:65, 0:65])
                        rec = rpool.tile([128, 1], F32, tag="rec",
                                         name=f"rec{h}_{t}")
                        nc.vector.reciprocal(rec[:], tp[:, 64:65])
                        nc.vector.tensor_scalar_mul(
                            osb[t][:, h * HD:(h + 1) * HD], tp[:, 0:64], rec[:])
                    nc.sync.dma_start(out[t * 128:(t + 1) * 128, :], osb[t][:])

    nc.compile()
    return nc


def _get_program(with_mask: bool, with_bias: bool):
    key = ("prog", with_mask, with_bias)
    if key not in _cache:
        _cache[key] = _build(with_mask, with_bias)
    return _cache[key]


def kernel(hidden_states, attention_mask, Wq, bq, Wk, bk, Wv, bv):
    x = np.asarray(hidden_states, np.float32).reshape(S, HID)
    mask = np.asarray(attention_mask, np.float32).reshape(-1)
    if mask.size == 1:
        mask = np.full(S, float(mask[0]), np.float32)
    with_mask = bool(np.any(mask))
    with_bias = bool(np.any(np.asarray(bq)) or np.any(np.asarray(bk))
                     or np.any(np.asarray(bv)))

    KIN = HID + 1 if with_bias else HID
    xtc = np.empty((KIN, S), np.float32)
    xtc[:HID] = x.T
    if with_bias:
        xtc[HID] = 1.0
    xtc = xtc.astype(ml_dtypes.bfloat16)

    # augmented weights: [3, KIN, 1024] with the bias as the last
    # contraction row; per-core slice is its pair's 128 output dims.
    w_aug = np.empty((3, KIN, HID), np.float32)
    for i, (W, b) in enumerate(((Wq, bq), (Wk, bk), (Wv, bv))):
        w_aug[i, :HID] = np.asarray(W, np.float32).T
        if with_bias:
            w_aug[i, HID] = np.asarray(b, np.float32)
    w_aug = w_aug.astype(ml_dtypes.bfloat16)

    ident = np.eye(128, dtype=np.float32)

    nc = _get_program(with_mask, with_bias)
    in_maps = []
    for c in range(N_CORES):
        m = {
            "xt": xtc,
            "w": np.ascontiguousarray(w_aug[:, :, c * PD:(c + 1) * PD]),
            "ident": ident,
        }
        if with_mask:
            m["maskt"] = np.ascontiguousarray(
                mask.reshape(KT, 128).T.astype(np.float32))
        in_maps.append(m)

    _cache["last_in_maps"] = in_maps
    _cache["last_prog"] = nc
    res = bass_utils.run_bass_kernel_spmd(nc, in_maps, core_ids=list(range(N_CORES)))
    out = np.concatenate([res.results[c]["out"] for c in range(N_CORES)], axis=1)
    return out.reshape(B, S, HID).astype(np.float32)


# revision 16
# speedup vs baseline: 1.2471x; 1.2471x over previous
"""Distributed self-attention kernel for Trainium2, 8 NeuronCores.

Head-parallel sharding (no collectives): with NH=16 heads on 8 cores,
each core owns one head PAIR (heads 2c, 2c+1 = hidden dims 128c..128c+128).
Every core loads the full x^T (6.3 MB bf16, column-chunked across DMA
queues so the first projection can start ~4us in), computes q/k/v
projections restricted to its pair's 128 output dims over the whole
sequence, runs attention for 2 heads x 3072 queries, and writes its
[3072, 128] slice of the hidden dim; the host concatenates.

The kernel is ScalarE-bound: exp() over 2x3072x3072 scores is ~153us of
ACT time that nothing else can absorb (DVE-side approximations fail the
accuracy budget: attention averaging passes per-element P noise through
to the output at full relative strength). Everything else is organized
to hide under that shadow:
  - scores run in transposed layout (s^T[key, query]); per-head k^T
    stationaries are zero-padded to the full 128-row contraction
    ([kE^T;0] / [0;kO^T]) so each head's scores are one full-rate matmul
    against the shared unpadded q block.
  - P@V stationary = a 128-col window of the per-key-tile v layout
    [v_even(64) | 1 | v_odd(64) | 1 | 0-pad(63)]; output rows 0-63 are
    context, row 64 the softmax denominator, rows 65-127 garbage.
  - exp on ScalarE, scale 1/sqrt(64) fused, no max subtraction (logits
    |qk/8| < ~4 — mathematically identical to the reference softmax).
  - the q/k/v projections AND the output transpose/normalize are
    interleaved into the attention sweep: key-tile groups are 2 wide so
    the score/ctx PSUM tiles leave 2 banks free (one rotating projection
    tile, one transpose tile). PSUM = 2+2+1+1+1+1 = 8 banks exactly.
Even/odd heads are staggered so ScalarE exp and PE matmuls pipeline.
"""

import numpy as np
import ml_dtypes

import concourse.bacc as bacc
import concourse.mybir as mybir
import concourse.tile as tile
from concourse import bass_utils

F32 = mybir.dt.float32
BF16 = mybir.dt.bfloat16
AF = mybir.ActivationFunctionType

N_CORES = 8
B, S, HID = 1, 3072, 1024
NH, HD = 16, 64
PD = 128                   # pair dims per core (2 heads x 64)
QC = 512                   # query chunk (moving cols per matmul)
NQC = S // QC              # 6 query chunks
KT = S // 128              # 24 key tiles
NG = KT // 2               # 12 groups of 2 key tiles
VTW = 193                  # per-kt v tile: vE(64)|1|vO(64)|1|zero-pad(63)
XCB = 768                  # x^T DMA column-chunk width

_cache: dict = {}


def _build(with_mask: bool, with_bias: bool):
    nc = bacc.Bacc("TRN2", target_bir_lowering=False, debug=False,
                   num_devices=N_CORES)

    J = 9 if with_bias else 8          # contraction slices (128 rows each)
    KIN = HID + 1 if with_bias else HID

    xt = nc.dram_tensor("xt", [KIN, S], BF16, kind="ExternalInput")
    w = nc.dram_tensor("w", [3, KIN, PD], BF16, kind="ExternalInput")
    ident = nc.dram_tensor("ident", [128, 128], F32, kind="ExternalInput")
    if with_mask:
        maskt = nc.dram_tensor("maskt", [128, KT], F32, kind="ExternalInput")
    out = nc.dram_tensor("out", [S, PD], F32, kind="ExternalOutput")

    with tile.TileContext(nc) as tc:
        with (
            tc.tile_pool(name="persist", bufs=1) as pp,
            tc.tile_pool(name="wpool", bufs=1) as wpool,
            tc.tile_pool(name="ppool", bufs=8) as ppool,
            tc.tile_pool(name="rpool", bufs=8) as rpool,
            # PSUM: 2+2+1+1+1+1 = 8 banks exactly
            tc.tile_pool(name="spoolE", bufs=1, space="PSUM") as spoolE,
            tc.tile_pool(name="spoolO", bufs=1, space="PSUM") as spoolO,
            tc.tile_pool(name="cpool", bufs=1, space="PSUM") as cpool,
            tc.tile_pool(name="pjpool", bufs=1, space="PSUM") as pjpool,
            tc.tile_pool(name="tpool", bufs=1, space="PSUM") as tpool,
        ):
            # ---- persistent SBUF tensors ----
            xsb = pp.tile([128, J * S], BF16)        # x^T contraction slices
            qsb = pp.tile([128, S], BF16)            # q^T pair block
            # k^T pair blocks, zero-padded per head so each head's scores
            # come out of one full-rate 128-contraction matmul:
            # ksbE = [kE^T; 0], ksbO = [0; kO^T]
            ksbE = pp.tile([128, S], BF16)
            ksbO = pp.tile([128, S], BF16)
            vsb = [pp.tile([128, VTW], BF16, name=f"vsb{k}") for k in range(KT)]
            idsb = pp.tile([128, 128], F32)
            ctxsb = [pp.tile([65, S], F32, name=f"ctxsb{h}") for h in range(2)]
            osb = [pp.tile([128, PD], F32, name=f"osb{t}") for t in range(KT)]

            # DMA trigger issue costs ~0.7us of sequencer time apiece, so
            # batch everything into a handful of strided-AP triggers and
            # split them between the Scalar and Sync queues.
            def load_w(proj):
                # all 8 contraction slices of one projection in one tile
                wt = pp.tile([128, 8 * PD], BF16, name=f"w{proj}")
                nc.scalar.dma_start(
                    wt[:].rearrange("p (j c) -> p j c", j=8),
                    w[proj, 0:HID, :].rearrange("(j p) c -> p j c", p=128))
                if with_bias:
                    wb = pp.tile([1, PD], BF16, name=f"wb{proj}")
                    nc.scalar.dma_start(wb[:], w[proj, HID:HID + 1, :])
                else:
                    wb = None
                return wt, wb

            (wq, wqb), (wk, wkb), (wv, wvb) = load_w(0), load_w(1), load_w(2)
            nc.scalar.dma_start(idsb[:], ident[:])
            if with_mask:
                msb = pp.tile([128, KT], F32)
                nc.scalar.dma_start(msb[:], maskt[:])

            # x^T load: one trigger per column block (early blocks land fast)
            for cb in range(S // XCB):
                nc.sync.dma_start(
                    xsb[:, 0:8 * S].rearrange("p (j c) -> p j c", j=8)
                    [:, :, cb * XCB:(cb + 1) * XCB],
                    xt[0:HID, cb * XCB:(cb + 1) * XCB]
                    .rearrange("(j p) c -> p j c", p=128))
            if with_bias:
                nc.sync.dma_start(xsb[0:1, 8 * S:9 * S], xt[1024:1025, :])

            nc.vector.memset(ksbE[64:128, :], 0.0)
            nc.vector.memset(ksbO[0:64, :], 0.0)
            for k in range(KT):
                nc.vector.memset(vsb[k][:, 130:VTW], 0.0)
                nc.vector.memset(vsb[k][:, 64:65], 1.0)
                nc.vector.memset(vsb[k][:, 129:130], 1.0)

            # ---- projection emitters (interleaved into the qc sweep) ----
            def proj_chunk(wt, wb, m, nm):
                # one 512-col chunk of k^T or q^T: [128 pair dims, 512 seq]
                pj = pjpool.tile([128, QC], F32, tag="pj", name=f"pj{nm}{m}")
                for j in range(8):
                    nc.tensor.matmul(
                        pj[:], wt[:, j * PD:(j + 1) * PD],
                        xsb[:, j * S + m * QC: j * S + (m + 1) * QC],
                        start=(j == 0), stop=(j == 7 and wb is None))
                if wb is not None:
                    nc.tensor.matmul(
                        pj[:], wb[0:1, :],
                        xsb[0:1, 8 * S + m * QC: 8 * S + (m + 1) * QC],
                        start=False, stop=True)
                return pj

            def k_chunk(m):
                pj = proj_chunk(wk, wkb, m, "k")
                nc.vector.tensor_copy(ksbE[0:64, m * QC:(m + 1) * QC],
                                      pj[0:64, :])
                nc.vector.tensor_copy(ksbO[64:128, m * QC:(m + 1) * QC],
                                      pj[64:128, :])

            def q_chunk(m):
                pj = proj_chunk(wq, wqb, m, "q")
                nc.vector.tensor_copy(qsb[:, m * QC:(m + 1) * QC], pj[:])

            def v_quad(qd):
                # v in [key, dim] layout: 4 key-tiles of [128 keys, 128 dims]
                # accumulated side by side in one rotating PSUM bank
                vt = pjpool.tile([128, QC], F32, tag="pj", name=f"pjv{qd}")
                for ktl in range(4):
                    kt = qd * 4 + ktl
                    for j in range(8):
                        nc.tensor.matmul(
                            vt[:, ktl * 128:(ktl + 1) * 128],
                            xsb[:, j * S + kt * 128: j * S + (kt + 1) * 128],
                            wv[:, j * PD:(j + 1) * PD],
                            start=(j == 0), stop=(j == 7 and wvb is None))
                    if wvb is not None:
                        nc.tensor.matmul(
                            vt[:, ktl * 128:(ktl + 1) * 128],
                            xsb[0:1, 8 * S + kt * 128: 8 * S + (kt + 1) * 128],
                            wvb[0:1, :],
                            start=False, stop=True)
                for ktl in range(4):
                    kt = qd * 4 + ktl
                    nc.vector.tensor_copy(
                        vsb[kt][:, 0:64], vt[:, ktl * 128: ktl * 128 + 64])
                    nc.vector.tensor_copy(
                        vsb[kt][:, 65:129], vt[:, ktl * 128 + 64: ktl * 128 + 128])

            # ---- attention sweep with A/D work woven in ----
            def score_block(sp, ksbh, qc, g):
                for j in range(2):
                    kt = g * 2 + j
                    nc.tensor.matmul(
                        sp[:, j * QC:(j + 1) * QC],
                        ksbh[:, kt * 128:(kt + 1) * 128],
                        qsb[:, qc * QC:(qc + 1) * QC],
                        start=True, stop=True)

            def exp_block(pt, sp, g):
                if with_mask:
                    for j in range(2):
                        kt = g * 2 + j
                        nc.scalar.activation(
                            pt[:, j * QC:(j + 1) * QC],
                            sp[:, j * QC:(j + 1) * QC], AF.Exp,
                            bias=msb[:, kt:kt + 1], scale=0.125)
                else:
                    nc.scalar.activation(pt[:], sp[:], AF.Exp, scale=0.125)

            def pv_block(ctx, pt, off, g):
                for j in range(2):
                    kt = g * 2 + j
                    nc.tensor.matmul(
                        ctx[:], vsb[kt][:, off:off + 128],
                        pt[:, j * QC:(j + 1) * QC],
                        start=(g == 0 and j == 0),
                        stop=(g == NG - 1 and j == 1))

            # prefix: first k / q chunks so the first exp fires early
            k_chunk(0)
            q_chunk(0)
            next_k, next_vq, next_q = 1, 0, 1

            for qc in range(NQC):
                ctxE = cpool.tile([128, QC], F32, tag="ctxE", name=f"cE{qc}")
                ctxO = cpool.tile([128, QC], F32, tag="ctxO", name=f"cO{qc}")
                for g in range(NG):
                    need = (2 * g + 1) // 4
                    while next_k <= need:
                        k_chunk(next_k)
                        next_k += 1
                    spE = spoolE.tile([128, 2 * QC], F32, tag="spE",
                                      name=f"spE{qc}_{g}")
                    score_block(spE, ksbE, qc, g)
                    spO = spoolO.tile([128, 2 * QC], F32, tag="spO",
                                      name=f"spO{qc}_{g}")
                    score_block(spO, ksbO, qc, g)
                    while next_vq <= need:
                        v_quad(next_vq)
                        next_vq += 1
                    ptE = ppool.tile([128, 2 * QC], BF16, tag="pt",
                                     name=f"ptE{qc}_{g}")
                    exp_block(ptE, spE, g)
                    ptO = ppool.tile([128, 2 * QC], BF16, tag="pt",
                                     name=f"ptO{qc}_{g}")
                    exp_block(ptO, spO, g)
                    pv_block(ctxE, ptE, 0, g)
                    pv_block(ctxO, ptO, 65, g)
                nc.vector.tensor_copy(ctxsb[0][:, qc * QC:(qc + 1) * QC],
                                      ctxE[0:65, :])
                nc.vector.tensor_copy(ctxsb[1][:, qc * QC:(qc + 1) * QC],
                                      ctxO[0:65, :])
                if next_q <= qc + 1 and next_q < NQC:
                    q_chunk(next_q)
                    next_q += 1

                # ---- D: transpose back, normalize, store (this qc) ----
                for t in range(qc * 4, (qc + 1) * 4):
                    tp = tpool.tile([128, 130], F32, tag="tp", name=f"tp{t}")
                    for h in range(2):
                        nc.tensor.transpose(
                            tp[:, h * 65:h * 65 + 65],
                            ctxsb[h][:, t * 128:(t + 1) * 128],
                            idsb[0:65, 0:65])
                    rec = rpool.tile([128, 2], F32, tag="rec", name=f"rec{t}")
                    nc.vector.reciprocal(
                        rec[:], tp[:].rearrange("p (h c) -> p h c", h=2)
                        [:, :, 64:65])
                    for h in range(2):
                        nc.vector.tensor_scalar_mul(
                            osb[t][:, h * HD:(h + 1) * HD],
                            tp[:, h * 65:h * 65 + 64], rec[:, h:h + 1])
                    nc.sync.dma_start(out[t * 128:(t + 1) * 128, :], osb[t][:])

    nc.compile()
    return nc


def _get_program(with_mask: bool, with_bias: bool):
    key = ("prog", with_mask, with_bias)
    if key not in _cache:
        _cache[key] = _build(with_mask, with_bias)
    return _cache[key]


def kernel(hidden_states, attention_mask, Wq, bq, Wk, bk, Wv, bv):
    x = np.asarray(hidden_states, np.float32).reshape(S, HID)
    mask = np.asarray(attention_mask, np.float32).reshape(-1)
    if mask.size == 1:
        mask = np.full(S, float(mask[0]), np.float32)
    with_mask = bool(np.any(mask))
    with_bias = bool(np.any(np.asarray(bq)) or np.any(np.asarray(bk))
                     or np.any(np.asarray(bv)))

    KIN = HID + 1 if with_bias else HID
    xtc = np.empty((KIN, S), np.float32)
    xtc[:HID] = x.T
    if with_bias:
        xtc[HID] = 1.0
    xtc = xtc.astype(ml_dtypes.bfloat16)

    # augmented weights: [3, KIN, 1024] with the bias as the last
    # contraction row; per-core slice is its pair's 128 output dims.
    w_aug = np.empty((3, KIN, HID), np.float32)
    for i, (W, b) in enumerate(((Wq, bq), (Wk, bk), (Wv, bv))):
        w_aug[i, :HID] = np.asarray(W, np.float32).T
        if with_bias:
            w_aug[i, HID] = np.asarray(b, np.float32)
    w_aug = w_aug.astype(ml_dtypes.bfloat16)

    ident = np.eye(128, dtype=np.float32)

    nc = _get_program(with_mask, with_bias)
    in_maps = []
    for c in range(N_CORES):
        m = {
            "xt": xtc,
            "w": np.ascontiguousarray(w_aug[:, :, c * PD:(c + 1) * PD]),
            "ident": ident,
        }
        if with_mask:
            m["maskt"] = np.ascontiguousarray(
                mask.reshape(KT, 128).T.astype(np.float32))
        in_maps.append(m)

    _cache["last_in_maps"] = in_maps
    _cache["last_prog"] = nc
    res = bass_utils.run_bass_kernel_spmd(nc, in_maps, core_ids=list(range(N_CORES)))
    out = np.concatenate([res.results[c]["out"] for c in range(N_CORES)], axis=1)
    return out.reshape(B, S, HID).astype(np.float32)


# revision 19
# speedup vs baseline: 1.2569x; 1.0078x over previous
"""Distributed self-attention kernel for Trainium2, 8 NeuronCores.

Head-parallel sharding (no collectives): with NH=16 heads on 8 cores,
each core owns one head PAIR (heads 2c, 2c+1 = hidden dims 128c..128c+128).
Every core loads the full x^T (6.3 MB bf16, column-chunked across DMA
queues so the first projection can start ~4us in), computes q/k/v
projections restricted to its pair's 128 output dims over the whole
sequence, runs attention for 2 heads x 3072 queries, and writes its
[3072, 128] slice of the hidden dim; the host concatenates.

The kernel is ScalarE-bound: exp() over 2x3072x3072 scores is ~153us of
ACT time that nothing else can absorb (DVE-side approximations fail the
accuracy budget: attention averaging passes per-element P noise through
to the output at full relative strength). Everything else is organized
to hide under that shadow:
  - scores run in transposed layout (s^T[key, query]); per-head k^T
    stationaries are zero-padded to the full 128-row contraction
    ([kE^T;0] / [0;kO^T]) so each head's scores are one full-rate matmul
    against the shared unpadded q block.
  - P@V stationary = a 128-col window of the per-key-tile v layout
    [v_even(64) | 1 | v_odd(64) | 1 | 0-pad(63)]; output rows 0-63 are
    context, row 64 the softmax denominator, rows 65-127 garbage.
  - exp on ScalarE, scale 1/sqrt(64) fused, no max subtraction (logits
    |qk/8| < ~4 — mathematically identical to the reference softmax).
  - the q/k/v projections AND the output transpose/normalize are
    interleaved into the attention sweep: key-tile groups are 2 wide so
    the score/ctx PSUM tiles leave 2 banks free (one rotating projection
    tile, one transpose tile). PSUM = 2+2+1+1+1+1 = 8 banks exactly.
Even/odd heads are staggered so ScalarE exp and PE matmuls pipeline.
"""

import numpy as np
import ml_dtypes

import concourse.bacc as bacc
import concourse.mybir as mybir
import concourse.tile as tile
from concourse import bass_utils

F32 = mybir.dt.float32
BF16 = mybir.dt.bfloat16
AF = mybir.ActivationFunctionType

N_CORES = 8
B, S, HID = 1, 3072, 1024
NH, HD = 16, 64
PD = 128                   # pair dims per core (2 heads x 64)
QC = 512                   # query chunk (moving cols per matmul)
NQC = S // QC              # 6 query chunks
KT = S // 128              # 24 key tiles
NG = KT // 2               # 12 groups of 2 key tiles
VTW = 193                  # per-kt v tile: vE(64)|1|vO(64)|1|zero-pad(63)
XCB = 768                  # x^T DMA column-chunk width

_cache: dict = {}


def _build(with_mask: bool, with_bias: bool):
    nc = bacc.Bacc("TRN2", target_bir_lowering=False, debug=False,
                   num_devices=N_CORES)

    J = 9 if with_bias else 8          # contraction slices (128 rows each)
    KIN = HID + 1 if with_bias else HID

    xt = nc.dram_tensor("xt", [KIN, S], BF16, kind="ExternalInput")
    w = nc.dram_tensor("w", [3, KIN, PD], BF16, kind="ExternalInput")
    ident = nc.dram_tensor("ident", [128, 128], F32, kind="ExternalInput")
    if with_mask:
        maskt = nc.dram_tensor("maskt", [128, KT], F32, kind="ExternalInput")
    out = nc.dram_tensor("out", [S, PD], F32, kind="ExternalOutput")

    with tile.TileContext(nc) as tc:
        with (
            tc.tile_pool(name="persist", bufs=1) as pp,
            tc.tile_pool(name="wpool", bufs=1) as wpool,
            tc.tile_pool(name="ppool", bufs=8) as ppool,
            tc.tile_pool(name="rpool", bufs=8) as rpool,
            # PSUM: 2+2+1+1+1+1 = 8 banks exactly
            tc.tile_pool(name="spoolE", bufs=1, space="PSUM") as spoolE,
            tc.tile_pool(name="spoolO", bufs=1, space="PSUM") as spoolO,
            tc.tile_pool(name="cpool", bufs=1, space="PSUM") as cpool,
            tc.tile_pool(name="pjpool", bufs=1, space="PSUM") as pjpool,
            tc.tile_pool(name="tpool", bufs=1, space="PSUM") as tpool,
        ):
            # ---- persistent SBUF tensors ----
            xsb = pp.tile([128, J * S], BF16)        # x^T contraction slices
            qsb = pp.tile([128, S], BF16)            # q^T pair block
            # k^T pair blocks, zero-padded per head so each head's scores
            # come out of one full-rate 128-contraction matmul:
            # ksbE = [kE^T; 0], ksbO = [0; kO^T]
            ksbE = pp.tile([128, S], BF16)
            ksbO = pp.tile([128, S], BF16)
            vsb = [pp.tile([128, VTW], BF16, name=f"vsb{k}") for k in range(KT)]
            idsb = pp.tile([128, 128], F32)
            ctxsb = [pp.tile([65, S], F32, name=f"ctxsb{h}") for h in range(2)]
            osb = pp.tile([128, KT * PD], F32)

            # DMA trigger issue costs ~0.7us of sequencer time apiece, so
            # batch everything into a handful of strided-AP triggers and
            # split them between the Scalar and Sync queues.
            def load_w(proj):
                # all 8 contraction slices of one projection in one tile
                wt = pp.tile([128, 8 * PD], BF16, name=f"w{proj}")
                nc.scalar.dma_start(
                    wt[:].rearrange("p (j c) -> p j c", j=8),
                    w[proj, 0:HID, :].rearrange("(j p) c -> p j c", p=128))
                if with_bias:
                    wb = pp.tile([1, PD], BF16, name=f"wb{proj}")
                    nc.scalar.dma_start(wb[:], w[proj, HID:HID + 1, :])
                else:
                    wb = None
                return wt, wb

            (wq, wqb), (wk, wkb), (wv, wvb) = load_w(0), load_w(1), load_w(2)
            nc.scalar.dma_start(idsb[:], ident[:])
            if with_mask:
                msb = pp.tile([128, KT], F32)
                nc.scalar.dma_start(msb[:], maskt[:])

            # x^T load in 4 column-block triggers alternating between the
            # Sync and Scalar DMA queues; the first (small) block unblocks
            # the k0/q0 projections within a few us.
            xv = xsb[:, 0:8 * S].rearrange("p (j c) -> p j c", j=8)
            for (c0, c1), eng in (((0, 512), nc.sync),
                                  ((512, 1024), nc.scalar),
                                  ((1024, 2048), nc.sync),
                                  ((2048, 3072), nc.scalar)):
                eng.dma_start(
                    xv[:, :, c0:c1],
                    xt[0:HID, c0:c1].rearrange("(j p) c -> p j c", p=128))
            if with_bias:
                nc.sync.dma_start(xsb[0:1, 8 * S:9 * S], xt[1024:1025, :])

            nc.vector.memset(ksbE[64:128, :], 0.0)
            nc.vector.memset(ksbO[0:64, :], 0.0)
            for k in range(KT):
                nc.vector.memset(vsb[k][:, 130:VTW], 0.0)
                nc.vector.memset(vsb[k][:, 64:65], 1.0)
                nc.vector.memset(vsb[k][:, 129:130], 1.0)

            # ---- projection emitters (interleaved into the qc sweep) ----
            def proj_chunk(wt, wb, m, nm):
                # one 512-col chunk of k^T or q^T: [128 pair dims, 512 seq]
                pj = pjpool.tile([128, QC], F32, tag="pj", name=f"pj{nm}{m}")
                for j in range(8):
                    nc.tensor.matmul(
                        pj[:], wt[:, j * PD:(j + 1) * PD],
                        xsb[:, j * S + m * QC: j * S + (m + 1) * QC],
                        start=(j == 0), stop=(j == 7 and wb is None))
                if wb is not None:
                    nc.tensor.matmul(
                        pj[:], wb[0:1, :],
                        xsb[0:1, 8 * S + m * QC: 8 * S + (m + 1) * QC],
                        start=False, stop=True)
                return pj

            def k_chunk(m):
                pj = proj_chunk(wk, wkb, m, "k")
                nc.vector.tensor_copy(ksbE[0:64, m * QC:(m + 1) * QC],
                                      pj[0:64, :])
                nc.vector.tensor_copy(ksbO[64:128, m * QC:(m + 1) * QC],
                                      pj[64:128, :])

            def q_chunk(m):
                pj = proj_chunk(wq, wqb, m, "q")
                nc.vector.tensor_copy(qsb[:, m * QC:(m + 1) * QC], pj[:])

            def v_quad(qd):
                # v in [key, dim] layout: 4 key-tiles of [128 keys, 128 dims]
                # accumulated side by side in one rotating PSUM bank
                vt = pjpool.tile([128, QC], F32, tag="pj", name=f"pjv{qd}")
                for ktl in range(4):
                    kt = qd * 4 + ktl
                    for j in range(8):
                        nc.tensor.matmul(
                            vt[:, ktl * 128:(ktl + 1) * 128],
                            xsb[:, j * S + kt * 128: j * S + (kt + 1) * 128],
                            wv[:, j * PD:(j + 1) * PD],
                            start=(j == 0), stop=(j == 7 and wvb is None))
                    if wvb is not None:
                        nc.tensor.matmul(
                            vt[:, ktl * 128:(ktl + 1) * 128],
                            xsb[0:1, 8 * S + kt * 128: 8 * S + (kt + 1) * 128],
                            wvb[0:1, :],
                            start=False, stop=True)
                for ktl in range(4):
                    kt = qd * 4 + ktl
                    nc.vector.tensor_copy(
                        vsb[kt][:, 0:64], vt[:, ktl * 128: ktl * 128 + 64])
                    nc.vector.tensor_copy(
                        vsb[kt][:, 65:129], vt[:, ktl * 128 + 64: ktl * 128 + 128])

            # ---- attention sweep with A/D work woven in ----
            def score_block(sp, ksbh, qc, g):
                for j in range(2):
                    kt = g * 2 + j
                    nc.tensor.matmul(
                        sp[:, j * QC:(j + 1) * QC],
                        ksbh[:, kt * 128:(kt + 1) * 128],
                        qsb[:, qc * QC:(qc + 1) * QC],
                        start=True, stop=True)

            def exp_block(pt, sp, g):
                if with_mask:
                    for j in range(2):
                        kt = g * 2 + j
                        nc.scalar.activation(
                            pt[:, j * QC:(j + 1) * QC],
                            sp[:, j * QC:(j + 1) * QC], AF.Exp,
                            bias=msb[:, kt:kt + 1], scale=0.125)
                else:
                    nc.scalar.activation(pt[:], sp[:], AF.Exp, scale=0.125)

            def pv_block(ctx, pt, off, g):
                for j in range(2):
                    kt = g * 2 + j
                    nc.tensor.matmul(
                        ctx[:], vsb[kt][:, off:off + 128],
                        pt[:, j * QC:(j + 1) * QC],
                        start=(g == 0 and j == 0),
                        stop=(g == NG - 1 and j == 1))

            # prefix: first k / q chunks so the first exp fires early
            k_chunk(0)
            q_chunk(0)
            next_k, next_vq, next_q = 1, 0, 1

            for qc in range(NQC):
                ctxE = cpool.tile([128, QC], F32, tag="ctxE", name=f"cE{qc}")
                ctxO = cpool.tile([128, QC], F32, tag="ctxO", name=f"cO{qc}")
                for g in range(NG):
                    need = (2 * g + 1) // 4
                    while next_k <= need:
                        k_chunk(next_k)
                        next_k += 1
                    spE = spoolE.tile([128, 2 * QC], F32, tag="spE",
                                      name=f"spE{qc}_{g}")
                    score_block(spE, ksbE, qc, g)
                    spO = spoolO.tile([128, 2 * QC], F32, tag="spO",
                                      name=f"spO{qc}_{g}")
                    score_block(spO, ksbO, qc, g)
                    while next_vq <= need:
                        v_quad(next_vq)
                        next_vq += 1
                    ptE = ppool.tile([128, 2 * QC], BF16, tag="pt",
                                     name=f"ptE{qc}_{g}")
                    exp_block(ptE, spE, g)
                    ptO = ppool.tile([128, 2 * QC], BF16, tag="pt",
                                     name=f"ptO{qc}_{g}")
                    exp_block(ptO, spO, g)
                    pv_block(ctxE, ptE, 0, g)
                    pv_block(ctxO, ptO, 65, g)
                nc.vector.tensor_copy(ctxsb[0][:, qc * QC:(qc + 1) * QC],
                                      ctxE[0:65, :])
                nc.vector.tensor_copy(ctxsb[1][:, qc * QC:(qc + 1) * QC],
                                      ctxO[0:65, :])
                if next_q <= qc + 1 and next_q < NQC:
                    q_chunk(next_q)
                    next_q += 1

                # ---- D: transpose back, normalize, store (this qc) ----
                for t in range(qc * 4, (qc + 1) * 4):
                    tp = tpool.tile([128, 130], F32, tag="tp", name=f"tp{t}")
                    for h in range(2):
                        nc.tensor.transpose(
                            tp[:, h * 65:h * 65 + 65],
                            ctxsb[h][:, t * 128:(t + 1) * 128],
                            idsb[0:65, 0:65])
                    rec = rpool.tile([128, 2], F32, tag="rec", name=f"rec{t}")
                    nc.vector.reciprocal(
                        rec[:], tp[:].rearrange("p (h c) -> p h c", h=2)
                        [:, :, 64:65])
                    for h in range(2):
                        nc.vector.tensor_scalar_mul(
                            osb[:, t * PD + h * HD: t * PD + (h + 1) * HD],
                            tp[:, h * 65:h * 65 + 64], rec[:, h:h + 1])
                nc.sync.dma_start(
                    out[qc * 512:(qc + 1) * 512, :]
                    .rearrange("(t p) c -> p t c", p=128),
                    osb[:, qc * 4 * PD:(qc + 1) * 4 * PD]
                    .rearrange("p (t c) -> p t c", t=4))

    nc.compile()
    return nc


def _get_program(with_mask: bool, with_bias: bool):
    key = ("prog", with_mask, with_bias)
    if key not in _cache:
        _cache[key] = _build(with_mask, with_bias)
    return _cache[key]


def kernel(hidden_states, attention_mask, Wq, bq, Wk, bk, Wv, bv):
    x = np.asarray(hidden_states, np.float32).reshape(S, HID)
    mask = np.asarray(attention_mask, np.float32).reshape(-1)
    if mask.size == 1:
        mask = np.full(S, float(mask[0]), np.float32)
    with_mask = bool(np.any(mask))
    with_bias = bool(np.any(np.asarray(bq)) or np.any(np.asarray(bk))
                     or np.any(np.asarray(bv)))

    KIN = HID + 1 if with_bias else HID
    xtc = np.empty((KIN, S), np.float32)
    xtc[:HID] = x.T
    if with_bias:
        xtc[HID] = 1.0
    xtc = xtc.astype(ml_dtypes.bfloat16)

    # augmented weights: [3, KIN, 1024] with the bias as the last
    # contraction row; per-core slice is its pair's 128 output dims.
    w_aug = np.empty((3, KIN, HID), np.float32)
    for i, (W, b) in enumerate(((Wq, bq), (Wk, bk), (Wv, bv))):
        w_aug[i, :HID] = np.asarray(W, np.float32).T
        if with_bias:
            w_aug[i, HID] = np.asarray(b, np.float32)
    w_aug = w_aug.astype(ml_dtypes.bfloat16)

    ident = np.eye(128, dtype=np.float32)

    nc = _get_program(with_mask, with_bias)
    in_maps = []
    for c in range(N_CORES):
        m = {
            "xt": xtc,
            "w": np.ascontiguousarray(w_aug[:, :, c * PD:(c + 1) * PD]),
            "ident": ident,
        }
        if with_mask:
            m["maskt"] = np.ascontiguousarray(
                mask.reshape(KT, 128).T.astype(np.float32))
        in_maps.append(m)

    _cache["last_in_maps"] = in_maps
    _cache["last_prog"] = nc
    res = bass_utils.run_bass_kernel_spmd(nc, in_maps, core_ids=list(range(N_CORES)))
    out = np.concatenate([res.results[c]["out"] for c in range(N_CORES)], axis=1)
    return out.reshape(B, S, HID).astype(np.float32)
